# revision 15
# baseline (speedup 1.0000x reference)
"""Trainium2 Bass kernel for nn_CustomLoss_74826920231413.

Loss structure (B=32, E=1024, K=20):
    c  = complex(nnOutput[:, :NOUT], nnOutput[:, NOUT:])
    d  = c[:, :K];  U = c[:, K:VLOC].reshape(B,E,K);  V = c[:, VLOC:].reshape(B,E,K)
    obj1/obj2 = sum_{j<k} |U^T U| / B (no conj), same for V
    pred = U @ diag(d) @ V^T;  tk = complex(kern_real, kern_imag)
    loss = ||tk - pred||^2 / ||tk||^2 + 0.01*(obj1+obj2)

Device strategy (data-parallel over B, 4 batch rows per core, 8 cores):
    ||tk - pred||^2 = ||tk||^2 - 2*Re<conj(tk),pred> + ||pred||^2, so the
    device only needs one streaming pass over tk producing small outputs:
      * gram[b]  = X^T X with X = [Ur|Ui|Vr|Vi]  (80x80)  -> objs + ||pred||^2
      * yr[b]    = W^T tkr with W = [Ur|Ui]      (40x1024) -> cross term
      * yi[b]    = W^T tki                        (40x1024)
      * den partials = per-partition sums of tk^2
    Host assembles the three scalars from these partials in float64.
"""

import sys

for _p in ("/opt/trn_rl_repo", "/root/.axon_site/_ro/trn_rl_repo"):
    if _p not in sys.path:
        sys.path.append(_p)

import numpy as np

import concourse.bacc as bacc
import concourse.bass as bass
import concourse.mybir as mybir
import concourse.tile as tile
from concourse.bass_utils import run_bass_kernel_spmd

# Problem constants (hardcoded per harness contract)
E = 1024
K = 20
NOUT = K * (2 * E + 1)          # 40980
VLOC = K + K * E                # 20500
PENALTY = 0.01
B = 32
NCORES = 8
NB = B // NCORES                # batch rows per core
NCH = E // 128                  # 8 e-chunks of 128 partitions
F32 = mybir.dt.float32
F32R = mybir.dt.float32r

_PROGRAM_CACHE = {}


def _build_program():
    """Per-core SPMD Bass program. Same program on all 8 cores; each core
    receives its own 4-row slice of the inputs."""
    nc = bacc.Bacc("TRN2", target_bir_lowering=False, debug=False)

    nn_d = nc.dram_tensor("nn", [NB, 2 * NOUT], F32, kind="ExternalInput").ap()
    tkr_d = nc.dram_tensor("tkr", [NB, E, E], F32, kind="ExternalInput").ap()
    tki_d = nc.dram_tensor("tki", [NB, E, E], F32, kind="ExternalInput").ap()

    gram_d = nc.dram_tensor("gram", [NB, 40, 80], F32, kind="ExternalOutput").ap()
    yr_d = nc.dram_tensor("yr", [NB, 40, E], F32, kind="ExternalOutput").ap()
    yi_d = nc.dram_tensor("yi", [NB, 40, E], F32, kind="ExternalOutput").ap()
    den_d = nc.dram_tensor("den", [2, 128, NB * NCH], F32, kind="ExternalOutput").ap()

    mult = mybir.AluOpType.mult
    Square = mybir.ActivationFunctionType.Square

    with tile.TileContext(nc) as tc:
        with (
            tc.tile_pool(name="xuv", bufs=2) as xpool,
            tc.tile_pool(name="tk", bufs=2) as tkpool,
            tc.tile_pool(name="scr", bufs=2) as scrpool,
            tc.tile_pool(name="evac", bufs=2) as evacpool,
            tc.tile_pool(name="den", bufs=1) as denpool,
            tc.tile_pool(name="psg", bufs=2, space="PSUM") as psg_pool,
            tc.tile_pool(name="psy", bufs=1, space="PSUM") as psy_pool,
        ):
            # den accumulator columns; each engine owns its own tile (no
            # cross-engine write conflicts). col = b*NCH + c
            den_dve = denpool.tile([128, NB * NCH], F32)
            den_act = denpool.tile([128, NB * NCH], F32)

            for b in range(NB):
                # ---- U and V tiles: x?[p, c, j, k], j in {0: real, 1: imag},
                # e = c*128 + p. Each DMA is a 3-dim pattern [p, c, k].
                xU = xpool.tile([128, NCH, 2, K], F32, name="xU")
                xV = xpool.tile([128, NCH, 2, K], F32, name="xV")
                for dst, off in (
                    (xU[:, :, 0], K),
                    (xU[:, :, 1], NOUT + K),
                    (xV[:, :, 0], VLOC),
                    (xV[:, :, 1], NOUT + VLOC),
                ):
                    src = nn_d[b, off:off + E * K].rearrange(
                        "(c p k) -> p c k", c=NCH, p=128, k=K
                    )
                    nc.sync.dma_start(dst, src)

                # ---- Y-matmul weights [Ur|Ui], typed fp32r at the DMA
                # producer so the BIR verifier accepts fp32r matmul operands
                w_sb = xpool.tile([128, NCH, 2, K], F32R, name="w_sb")
                for j, off in enumerate((K, NOUT + K)):
                    src = nn_d[b, off:off + E * K].rearrange(
                        "(c p k) -> p c k", c=NCH, p=128, k=K
                    )
                    nc.sync.dma_start(w_sb[:, :, j], src.bitcast(F32R))

                # ---- stream in this row's kernels: [p, c, f], e = c*128+p
                tkr_sb = tkpool.tile([128, NCH, E], F32R, name="tkr_sb")
                nc.sync.dma_start(
                    tkr_sb[:],
                    tkr_d[b].rearrange("(c p) f -> p c f", c=NCH, p=128).bitcast(F32R),
                )
                tki_sb = tkpool.tile([128, NCH, E], F32R, name="tki_sb")
                nc.sync.dma_start(
                    tki_sb[:],
                    tki_d[b].rearrange("(c p) f -> p c f", c=NCH, p=128).bitcast(F32R),
                )

                # ---- Grams: S_U = [Ur|Ui]^T [Ur|Ui], S_V likewise (exact
                # fp32).  The U-V cross blocks are never needed by the host.
                ps_g = psg_pool.tile([40, 80], F32, name="ps_g")
                for c in range(NCH):
                    xu = xU[:, c].rearrange("p j k -> p (j k)")
                    nc.tensor.matmul(
                        ps_g[:, 0:40], xu, xu, start=(c == 0), stop=(c == NCH - 1)
                    )
                for c in range(NCH):
                    xv = xV[:, c].rearrange("p j k -> p (j k)")
                    nc.tensor.matmul(
                        ps_g[:, 40:80], xv, xv, start=(c == 0), stop=(c == NCH - 1)
                    )
                g_sb = evacpool.tile([40, 80], F32, name="g_sb")
                nc.vector.tensor_copy(g_sb[:], ps_g[:])
                nc.sync.dma_start(gram_d[b], g_sb[:])

                # ---- Y: yr[j,f] = sum_e W[e,j] tkr[e,f], W = [Ur|Ui]
                # (fp32r: full-rate streaming; precision ample for the
                # cross term, which perturbs num at ~1e-7 relative)
                ps_yr = psy_pool.tile([40, E], F32, name="ps_yr")
                ps_yi = psy_pool.tile([40, E], F32, name="ps_yi")
                for c in range(NCH):
                    w = w_sb[:, c].rearrange("p j k -> p (j k)")
                    for h in range(2):
                        fs = slice(h * 512, (h + 1) * 512)
                        nc.tensor.matmul(
                            ps_yr[:, fs],
                            w,
                            tkr_sb[:, c, fs],
                            start=(c == 0),
                            stop=(c == NCH - 1),
                        )
                        nc.tensor.matmul(
                            ps_yi[:, fs],
                            w,
                            tki_sb[:, c, fs],
                            start=(c == 0),
                            stop=(c == NCH - 1),
                        )
                yr_sb = evacpool.tile([40, E], F32, name="yr_sb")
                nc.vector.tensor_copy(yr_sb[:], ps_yr[:])
                nc.sync.dma_start(yr_d[b], yr_sb[:])
                yi_sb = evacpool.tile([40, E], F32, name="yi_sb")
                nc.vector.tensor_copy(yi_sb[:], ps_yi[:])
                nc.sync.dma_start(yi_d[b], yi_sb[:])

                # ---- den partials: sum of squares along free dim.
                # tkr chunks on DVE, tki chunks on ACT (load balance).
                for c in range(NCH):
                    col = b * NCH + c
                    tkr_f32 = tkr_sb[:, c, :].bitcast(F32)
                    scr_v = scrpool.tile([128, E], F32, name="scr_v")
                    nc.vector.scalar_tensor_tensor(
                        scr_v[:],
                        tkr_f32,
                        1.0,
                        tkr_f32,
                        mult,
                        mult,
                        accum_out=den_dve[:, col:col + 1],
                    )
                    scr_a = scrpool.tile([128, E], F32, name="scr_a")
                    nc.scalar.activation(
                        scr_a[:],
                        tki_sb[:, c, :].bitcast(F32),
                        Square,
                        accum_out=den_act[:, col:col + 1],
                    )

            nc.sync.dma_start(den_d[0], den_dve[:])
            nc.sync.dma_start(den_d[1], den_act[:])

    nc.compile()
    return nc


def _get_program():
    if "nc" not in _PROGRAM_CACHE:
        _PROGRAM_CACHE["nc"] = _build_program()
    return _PROGRAM_CACHE["nc"]


def _run_device(nn, tkr, tki, trace=False):
    nc = _get_program()
    in_maps = [
        {
            "nn": np.ascontiguousarray(nn[i * NB:(i + 1) * NB]),
            "tkr": np.ascontiguousarray(tkr[i * NB:(i + 1) * NB]),
            "tki": np.ascontiguousarray(tki[i * NB:(i + 1) * NB]),
        }
        for i in range(NCORES)
    ]
    return run_bass_kernel_spmd(nc, in_maps, list(range(NCORES)), trace=trace)


def _finalize(nn, results, batch_size):
    """Assemble (loss, obj1, obj2) from per-core device partials (float64)."""
    nn = np.asarray(nn)
    d = (nn[:, :K] + 1j * nn[:, NOUT:NOUT + K]).astype(np.complex128)
    Vr = nn[:, VLOC:NOUT].reshape(B, E, K).astype(np.float64)
    Vi = nn[:, NOUT + VLOC:2 * NOUT].reshape(B, E, K).astype(np.float64)
    V = Vr + 1j * Vi

    gram = np.concatenate(
        [r["gram"] for r in results], axis=0
    ).astype(np.float64)                                   # [B, 40, 80]
    yr = np.concatenate([r["yr"] for r in results], axis=0).astype(np.float64)
    yi = np.concatenate([r["yi"] for r in results], axis=0).astype(np.float64)
    den = float(sum(np.sum(r["den"], dtype=np.float64) for r in results))

    SU = gram[:, :, 0:40]
    SV = gram[:, :, 40:80]
    Srr = SU[:, 0:20, 0:20]
    Sri = SU[:, 0:20, 20:40]
    Sii = SU[:, 20:40, 20:40]
    Trr = SV[:, 0:20, 0:20]
    Tri = SV[:, 0:20, 20:40]
    Tii = SV[:, 20:40, 20:40]
    SriT = np.transpose(Sri, (0, 2, 1))
    TriT = np.transpose(Tri, (0, 2, 1))
    G_U = (Srr - Sii) + 1j * (Sri + SriT)
    G_V = (Trr - Tii) + 1j * (Tri + TriT)
    H_U = (Srr + Sii) + 1j * (Sri - SriT)
    H_V = (Trr + Tii) + 1j * (Tri - TriT)

    mask = np.triu(np.ones((K, K), dtype=bool), k=1)
    bsz = float(batch_size)
    obj1 = float(np.sum(np.abs(G_U)[:, mask]) / bsz)
    obj2 = float(np.sum(np.abs(G_V)[:, mask]) / bsz)

    prednorm = float(
        np.real(
            np.einsum("bk,bl,bkl,bkl->", d, np.conj(d), np.conj(H_U), np.conj(H_V))
        )
    )

    # cross = Re<conj(tk), pred>; Wc[b,k,f] = sum_e conj(tk[e,f]) U[e,k]
    Wc = (yr[:, 0:20, :] + yi[:, 20:40, :]) + 1j * (yr[:, 20:40, :] - yi[:, 0:20, :])
    zeta = np.einsum("bfk,bkf->bk", V, Wc)
    cross = float(np.real(np.einsum("bk,bk->", d, zeta)))

    num = den - 2.0 * cross + prednorm
    loss = num / den + PENALTY * (obj1 + obj2)
    return (
        np.float32(loss),
        np.float32(obj1),
        np.float32(obj2),
    )


def kernel(nnOutput, kern_real, kern_imag, batch_Size):
    nn = np.ascontiguousarray(np.asarray(nnOutput, dtype=np.float32))
    tkr = np.asarray(kern_real, dtype=np.float32)
    tki = np.asarray(kern_imag, dtype=np.float32)
    res = _run_device(nn, tkr, tki).results
    return _finalize(nn, res, int(batch_Size))


# revision 16
# speedup vs baseline: 1.4242x; 1.4242x over previous
"""Trainium2 Bass kernel for nn_CustomLoss_74826920231413.

Loss structure (B=32, E=1024, K=20):
    c  = complex(nnOutput[:, :NOUT], nnOutput[:, NOUT:])
    d  = c[:, :K];  U = c[:, K:VLOC].reshape(B,E,K);  V = c[:, VLOC:].reshape(B,E,K)
    obj1/obj2 = sum_{j<k} |U^T U| / B (no conj), same for V
    pred = U @ diag(d) @ V^T;  tk = complex(kern_real, kern_imag)
    loss = ||tk - pred||^2 / ||tk||^2 + 0.01*(obj1+obj2)

Device strategy (data-parallel over B, 4 batch rows per core, 8 cores):
    ||tk - pred||^2 = ||tk||^2 - 2*Re<conj(tk),pred> + ||pred||^2, so the
    device only needs one streaming pass over tk producing small outputs:
      * gram[b]  = [Ur|Ui]^T[Ur|Ui] and [Vr|Vi]^T[Vr|Vi]  -> objs, ||pred||^2
      * yr[b]    = W^T tkr with W = [Ur|Ui]      (40x1024) -> cross term
      * yi[b]    = W^T tki                        (40x1024)
      * den partials = per-partition sums of tk^2
    Host assembles the three scalars from these partials in float64.

    tk is shipped to the device as fp16: the loss is a ratio of O(1e9)
    quantities and 16-bit rounding of tk perturbs it at ~1e-6 relative
    (validated numerically), while halving the dominant DMA traffic.
    Gram runs in exact fp32 from the fp32 nnOutput. tkr streams on the
    sync HWDGE ring, tki on the scalar HWDGE ring; small transfers ride
    gpsimd SWDGE queues.
"""

import sys

for _p in ("/opt/trn_rl_repo", "/root/.axon_site/_ro/trn_rl_repo"):
    if _p not in sys.path:
        sys.path.append(_p)

import numpy as np

import concourse.bacc as bacc
import concourse.mybir as mybir
import concourse.tile as tile
from concourse.bass_utils import run_bass_kernel_spmd

# Problem constants (hardcoded per harness contract)
E = 1024
K = 20
NOUT = K * (2 * E + 1)          # 40980
VLOC = K + K * E                # 20500
PENALTY = 0.01
B = 32
NCORES = 8
NB = B // NCORES                # batch rows per core
NCH = E // 128                  # 8 e-chunks of 128 partitions
HALF = NCH // 2                 # tk DMA split granularity (chunks per DMA)
F32 = mybir.dt.float32
F16 = mybir.dt.float16

_PROGRAM_CACHE = {}


def _build_program():
    """Per-core SPMD Bass program. Same program on all 8 cores; each core
    receives its own 4-row slice of the inputs (host-packed layouts)."""
    nc = bacc.Bacc("TRN2", target_bir_lowering=False, debug=False)

    # host-packed [Ur|Ui|Vr|Vi] fp32, partition-major: [b, p, c, 80]
    xuv_d = nc.dram_tensor("xuv", [NB, 128, NCH, 80], F32, kind="ExternalInput").ap()
    # host-packed fp16 [Ur|Ui] weights: [b, p, c, 40]
    w_d = nc.dram_tensor("w16", [NB, 128, NCH, 40], F16, kind="ExternalInput").ap()
    tkr_d = nc.dram_tensor("tkr", [NB, E, E], F16, kind="ExternalInput").ap()
    tki_d = nc.dram_tensor("tki", [NB, E, E], F16, kind="ExternalInput").ap()

    gram_d = nc.dram_tensor("gram", [NB, 40, 80], F32, kind="ExternalOutput").ap()
    yr_d = nc.dram_tensor("yr", [NB, 40, E], F32, kind="ExternalOutput").ap()
    yi_d = nc.dram_tensor("yi", [NB, 40, E], F32, kind="ExternalOutput").ap()
    den_d = nc.dram_tensor("den", [2, 128, NB * NCH], F32, kind="ExternalOutput").ap()

    mult = mybir.AluOpType.mult
    Square = mybir.ActivationFunctionType.Square

    with tile.TileContext(nc) as tc:
        with (
            tc.tile_pool(name="xuv", bufs=2) as xpool,
            tc.tile_pool(name="tk", bufs=3) as tkpool,
            tc.tile_pool(name="scr", bufs=2) as scrpool,
            tc.tile_pool(name="evac", bufs=2) as evacpool,
            tc.tile_pool(name="den", bufs=1) as denpool,
            tc.tile_pool(name="psg", bufs=2, space="PSUM") as psg_pool,
            tc.tile_pool(name="psy", bufs=1, space="PSUM") as psy_pool,
        ):
            # den accumulator columns; each engine owns its own tile (no
            # cross-engine write conflicts). col = b*NCH + c
            den_dve = denpool.tile([128, NB * NCH], F32, name="den_dve")
            den_act = denpool.tile([128, NB * NCH], F32, name="den_act")

            for b in range(NB):
                # ---- kernels, fp16, halves for pipelining: [p, c, f]
                tkr_sb = []
                tki_sb = []
                for h in range(NCH // HALF):
                    cs = slice(h * HALF * 128, (h + 1) * HALF * 128)
                    tr = tkpool.tile([128, HALF, E], F16, name=f"tkr_h{h}")
                    nc.sync.dma_start(
                        tr[:],
                        tkr_d[b, cs].rearrange("(c p) f -> p c f", c=HALF, p=128),
                    )
                    tkr_sb.append(tr)
                    ti = tkpool.tile([128, HALF, E], F16, name=f"tki_h{h}")
                    nc.scalar.dma_start(
                        ti[:],
                        tki_d[b, cs].rearrange("(c p) f -> p c f", c=HALF, p=128),
                    )
                    tki_sb.append(ti)

                def tkr_c(c):
                    return tkr_sb[c // HALF][:, c % HALF, :]

                def tki_c(c):
                    return tki_sb[c // HALF][:, c % HALF, :]

                # ---- U/V tile (fp32) + fp16 Y weights, host-packed layouts
                x_sb = xpool.tile([128, NCH, 80], F32, name="x_sb")
                nc.gpsimd.dma_start(x_sb[:], xuv_d[b])
                w_sb = xpool.tile([128, NCH, 40], F16, name="w_sb")
                nc.gpsimd.dma_start(w_sb[:], w_d[b])

                # ---- Grams: S_U = [Ur|Ui]^T [Ur|Ui], S_V likewise (exact
                # fp32). The U-V cross blocks are never needed by the host.
                ps_g = psg_pool.tile([40, 80], F32, name="ps_g")
                for c in range(NCH):
                    xu = x_sb[:, c, 0:40]
                    nc.tensor.matmul(
                        ps_g[:, 0:40], xu, xu, start=(c == 0), stop=(c == NCH - 1)
                    )
                for c in range(NCH):
                    xv = x_sb[:, c, 40:80]
                    nc.tensor.matmul(
                        ps_g[:, 40:80], xv, xv, start=(c == 0), stop=(c == NCH - 1)
                    )
                g_sb = evacpool.tile([40, 80], F32, name="g_sb")
                nc.vector.tensor_copy(g_sb[:], ps_g[:])
                nc.gpsimd.dma_start(gram_d[b], g_sb[:])

                # ---- Y: yr[j,f] = sum_e W[e,j] tkr[e,f], W = [Ur|Ui] (fp16)
                ps_yr = psy_pool.tile([40, E], F32, name="ps_yr")
                ps_yi = psy_pool.tile([40, E], F32, name="ps_yi")
                for c in range(NCH):
                    w = w_sb[:, c, :]
                    for h in range(2):
                        fs = slice(h * 512, (h + 1) * 512)
                        nc.tensor.matmul(
                            ps_yr[:, fs],
                            w,
                            tkr_c(c)[:, fs],
                            start=(c == 0),
                            stop=(c == NCH - 1),
                        )
                        nc.tensor.matmul(
                            ps_yi[:, fs],
                            w,
                            tki_c(c)[:, fs],
                            start=(c == 0),
                            stop=(c == NCH - 1),
                        )
                yr_sb = evacpool.tile([40, E], F32, name="yr_sb")
                nc.vector.tensor_copy(yr_sb[:], ps_yr[:])
                nc.gpsimd.dma_start(yr_d[b], yr_sb[:])
                yi_sb = evacpool.tile([40, E], F32, name="yi_sb")
                nc.vector.tensor_copy(yi_sb[:], ps_yi[:])
                nc.gpsimd.dma_start(yi_d[b], yi_sb[:])

                # ---- den partials: sum of squares along free dim (fp32
                # accumulate). tkr chunks on DVE, tki chunks on ACT.
                for c in range(NCH):
                    col = b * NCH + c
                    scr_v = scrpool.tile([128, E], F16, name="scr_v")
                    nc.vector.scalar_tensor_tensor(
                        scr_v[:],
                        tkr_c(c),
                        1.0,
                        tkr_c(c),
                        mult,
                        mult,
                        accum_out=den_dve[:, col:col + 1],
                    )
                    scr_a = scrpool.tile([128, E], F16, name="scr_a")
                    nc.scalar.activation(
                        scr_a[:],
                        tki_c(c),
                        Square,
                        accum_out=den_act[:, col:col + 1],
                    )

            nc.gpsimd.dma_start(den_d[0], den_dve[:])
            nc.gpsimd.dma_start(den_d[1], den_act[:])

    nc.compile()
    return nc


def _get_program():
    if "nc" not in _PROGRAM_CACHE:
        _PROGRAM_CACHE["nc"] = _build_program()
    return _PROGRAM_CACHE["nc"]


def _pack_inputs(nn, tkr, tki):
    """Host-side packing: per-core input dicts with device-friendly layouts."""
    tkr16 = tkr.astype(np.float16)
    tki16 = tki.astype(np.float16)
    # [B, E, K] slices of nn
    Ur = nn[:, K:VLOC].reshape(B, E, K)
    Ui = nn[:, NOUT + K:NOUT + VLOC].reshape(B, E, K)
    Vr = nn[:, VLOC:NOUT].reshape(B, E, K)
    Vi = nn[:, NOUT + VLOC:2 * NOUT].reshape(B, E, K)
    xuv = np.concatenate([Ur, Ui, Vr, Vi], axis=2)        # [B, E, 80] f32
    # partition-major: e = c*128 + p  ->  [B, p, c, 80]
    xuv = np.ascontiguousarray(
        xuv.reshape(B, NCH, 128, 80).transpose(0, 2, 1, 3)
    )
    w16 = np.ascontiguousarray(
        np.concatenate([Ur, Ui], axis=2)
        .reshape(B, NCH, 128, 40)
        .transpose(0, 2, 1, 3)
        .astype(np.float16)
    )
    return [
        {
            "xuv": xuv[i * NB:(i + 1) * NB],
            "w16": w16[i * NB:(i + 1) * NB],
            "tkr": tkr16[i * NB:(i + 1) * NB],
            "tki": tki16[i * NB:(i + 1) * NB],
        }
        for i in range(NCORES)
    ]


def _run_device(nn, tkr, tki, trace=False):
    nc = _get_program()
    in_maps = _pack_inputs(nn, tkr, tki)
    return run_bass_kernel_spmd(nc, in_maps, list(range(NCORES)), trace=trace)


def _finalize(nn, results, batch_size):
    """Assemble (loss, obj1, obj2) from per-core device partials (float64)."""
    nn = np.asarray(nn)
    d = (nn[:, :K] + 1j * nn[:, NOUT:NOUT + K]).astype(np.complex128)
    Vr = nn[:, VLOC:NOUT].reshape(B, E, K).astype(np.float64)
    Vi = nn[:, NOUT + VLOC:2 * NOUT].reshape(B, E, K).astype(np.float64)
    V = Vr + 1j * Vi

    gram = np.concatenate(
        [r["gram"] for r in results], axis=0
    ).astype(np.float64)                                   # [B, 40, 80]
    yr = np.concatenate([r["yr"] for r in results], axis=0).astype(np.float64)
    yi = np.concatenate([r["yi"] for r in results], axis=0).astype(np.float64)
    den = float(sum(np.sum(r["den"], dtype=np.float64) for r in results))

    SU = gram[:, :, 0:40]
    SV = gram[:, :, 40:80]
    Srr = SU[:, 0:20, 0:20]
    Sri = SU[:, 0:20, 20:40]
    Sii = SU[:, 20:40, 20:40]
    Trr = SV[:, 0:20, 0:20]
    Tri = SV[:, 0:20, 20:40]
    Tii = SV[:, 20:40, 20:40]
    SriT = np.transpose(Sri, (0, 2, 1))
    TriT = np.transpose(Tri, (0, 2, 1))
    G_U = (Srr - Sii) + 1j * (Sri + SriT)
    G_V = (Trr - Tii) + 1j * (Tri + TriT)
    H_U = (Srr + Sii) + 1j * (Sri - SriT)
    H_V = (Trr + Tii) + 1j * (Tri - TriT)

    mask = np.triu(np.ones((K, K), dtype=bool), k=1)
    bsz = float(batch_size)
    obj1 = float(np.sum(np.abs(G_U)[:, mask]) / bsz)
    obj2 = float(np.sum(np.abs(G_V)[:, mask]) / bsz)

    prednorm = float(
        np.real(
            np.einsum("bk,bl,bkl,bkl->", d, np.conj(d), np.conj(H_U), np.conj(H_V))
        )
    )

    # cross = Re<conj(tk), pred>; Wc[b,k,f] = sum_e conj(tk[e,f]) U[e,k]
    Wc = (yr[:, 0:20, :] + yi[:, 20:40, :]) + 1j * (yr[:, 20:40, :] - yi[:, 0:20, :])
    zeta = np.einsum("bfk,bkf->bk", V, Wc)
    cross = float(np.real(np.einsum("bk,bk->", d, zeta)))

    num = den - 2.0 * cross + prednorm
    loss = num / den + PENALTY * (obj1 + obj2)
    return (
        np.float32(loss),
        np.float32(obj1),
        np.float32(obj2),
    )


def kernel(nnOutput, kern_real, kern_imag, batch_Size):
    nn = np.ascontiguousarray(np.asarray(nnOutput, dtype=np.float32))
    tkr = np.asarray(kern_real, dtype=np.float32)
    tki = np.asarray(kern_imag, dtype=np.float32)
    res = _run_device(nn, tkr, tki).results
    return _finalize(nn, res, int(batch_Size))


# revision 19
# speedup vs baseline: 1.7159x; 1.2049x over previous
"""Trainium2 Bass kernel for nn_CustomLoss_74826920231413.

Loss structure (B=32, E=1024, K=20):
    c  = complex(nnOutput[:, :NOUT], nnOutput[:, NOUT:])
    d  = c[:, :K];  U = c[:, K:VLOC].reshape(B,E,K);  V = c[:, VLOC:].reshape(B,E,K)
    obj1/obj2 = sum_{j<k} |U^T U| / B (no conj), same for V
    pred = U @ diag(d) @ V^T;  tk = complex(kern_real, kern_imag)
    loss = ||tk - pred||^2 / ||tk||^2 + 0.01*(obj1+obj2)

Device strategy (data-parallel over B, 4 batch rows per core, 8 cores):
    ||tk - pred||^2 = ||tk||^2 - 2*Re<conj(tk),pred> + ||pred||^2, so the
    device only needs one streaming pass over tk producing small outputs:
      * gram[b]  = [Ur|Ui]^T[Ur|Ui] and [Vr|Vi]^T[Vr|Vi]  -> objs, ||pred||^2
      * yr[b]    = W^T tkr with W = [Ur|Ui]      (40x1024) -> cross term
      * yi[b]    = W^T tki                        (40x1024)
      * den partials = per-partition sums of tk^2
    Host assembles the three scalars from these partials in float64.

    tk is shipped to the device as fp16: the loss is a ratio of O(1e9)
    quantities and 16-bit rounding of tk perturbs it at ~1e-6 relative
    (validated numerically), while halving the dominant DMA traffic.
    Gram runs in exact fp32 from the fp32 nnOutput. tkr streams on the
    sync HWDGE ring, tki on the scalar HWDGE ring; small transfers ride
    gpsimd SWDGE queues.
"""

import sys

for _p in ("/opt/trn_rl_repo", "/root/.axon_site/_ro/trn_rl_repo"):
    if _p not in sys.path:
        sys.path.append(_p)

import numpy as np

import concourse.bacc as bacc
import concourse.mybir as mybir
import concourse.tile as tile
from concourse.bass_utils import run_bass_kernel_spmd

# Problem constants (hardcoded per harness contract)
E = 1024
K = 20
NOUT = K * (2 * E + 1)          # 40980
VLOC = K + K * E                # 20500
PENALTY = 0.01
B = 32
NCORES = 8
NB = B // NCORES                # batch rows per core
NCH = E // 128                  # 8 e-chunks of 128 partitions
HALF = NCH // 2                 # tk DMA split granularity (chunks per DMA)
F32 = mybir.dt.float32
F16 = mybir.dt.float16

_PROGRAM_CACHE = {}


def _build_program():
    """Per-core SPMD Bass program. Same program on all 8 cores; each core
    receives its own 4-row slice of the inputs (host-packed layouts)."""
    nc = bacc.Bacc("TRN2", target_bir_lowering=False, debug=False)

    # host-packed [Ur|Ui|Vr|Vi] fp32, partition-major: [b, p, c, 80]
    xuv_d = nc.dram_tensor("xuv", [NB, 128, NCH, 80], F32, kind="ExternalInput").ap()
    # host-packed fp16 [Ur|Ui] weights: [b, p, c, 40]
    w_d = nc.dram_tensor("w16", [NB, 128, NCH, 40], F16, kind="ExternalInput").ap()
    tkr_d = nc.dram_tensor("tkr", [NB, E, E], F16, kind="ExternalInput").ap()
    tki_d = nc.dram_tensor("tki", [NB, E, E], F16, kind="ExternalInput").ap()

    gram_d = nc.dram_tensor("gram", [NB, 40, 80], F32, kind="ExternalOutput").ap()
    yr_d = nc.dram_tensor("yr", [NB, 40, E], F32, kind="ExternalOutput").ap()
    yi_d = nc.dram_tensor("yi", [NB, 40, E], F32, kind="ExternalOutput").ap()
    den_d = nc.dram_tensor("den", [2, 128, NB * NCH], F32, kind="ExternalOutput").ap()

    mult = mybir.AluOpType.mult
    Square = mybir.ActivationFunctionType.Square

    with tile.TileContext(nc) as tc:
        with (
            tc.tile_pool(name="xuv", bufs=2) as xpool,
            tc.tile_pool(name="tk", bufs=3) as tkpool,
            tc.tile_pool(name="scr", bufs=2) as scrpool,
            tc.tile_pool(name="evac", bufs=2) as evacpool,
            tc.tile_pool(name="den", bufs=1) as denpool,
            tc.tile_pool(name="psg", bufs=2, space="PSUM") as psg_pool,
            tc.tile_pool(name="psy", bufs=1, space="PSUM") as psy_pool,
        ):
            # den accumulator columns; each engine owns its own tile (no
            # cross-engine write conflicts). col = b*NCH + c
            den_dve = denpool.tile([128, NB * NCH], F32, name="den_dve")
            den_act = denpool.tile([128, NB * NCH], F32, name="den_act")

            for b in range(NB):
                # ---- kernels, fp16, halves for pipelining: [p, c, f]
                tkr_sb = []
                tki_sb = []
                for h in range(NCH // HALF):
                    cs = slice(h * HALF * 128, (h + 1) * HALF * 128)
                    tr = tkpool.tile([128, HALF, E], F16, name=f"tkr_h{h}")
                    nc.sync.dma_start(
                        tr[:],
                        tkr_d[b, cs].rearrange("(c p) f -> p c f", c=HALF, p=128),
                    )
                    tkr_sb.append(tr)
                    ti = tkpool.tile([128, HALF, E], F16, name=f"tki_h{h}")
                    nc.sync.dma_start(
                        ti[:],
                        tki_d[b, cs].rearrange("(c p) f -> p c f", c=HALF, p=128),
                    )
                    tki_sb.append(ti)

                def tkr_c(c):
                    return tkr_sb[c // HALF][:, c % HALF, :]

                def tki_c(c):
                    return tki_sb[c // HALF][:, c % HALF, :]

                # ---- U/V tile (fp32) + fp16 Y weights, host-packed layouts
                x_sb = xpool.tile([128, NCH, 80], F32, name="x_sb")
                nc.gpsimd.dma_start(x_sb[:], xuv_d[b])
                w_sb = xpool.tile([128, NCH, 40], F16, name="w_sb")
                nc.gpsimd.dma_start(w_sb[:], w_d[b])

                # ---- Grams: S_U = [Ur|Ui]^T [Ur|Ui], S_V likewise (exact
                # fp32). The U-V cross blocks are never needed by the host.
                ps_g = psg_pool.tile([40, 80], F32, name="ps_g")
                for c in range(NCH):
                    xu = x_sb[:, c, 0:40]
                    nc.tensor.matmul(
                        ps_g[:, 0:40], xu, xu, start=(c == 0), stop=(c == NCH - 1)
                    )
                for c in range(NCH):
                    xv = x_sb[:, c, 40:80]
                    nc.tensor.matmul(
                        ps_g[:, 40:80], xv, xv, start=(c == 0), stop=(c == NCH - 1)
                    )
                g_sb = evacpool.tile([40, 80], F32, name="g_sb")
                nc.vector.tensor_copy(g_sb[:], ps_g[:])
                nc.gpsimd.dma_start(gram_d[b], g_sb[:])

                # ---- Y: yr[j,f] = sum_e W[e,j] tkr[e,f], W = [Ur|Ui] (fp16)
                ps_yr = psy_pool.tile([40, E], F32, name="ps_yr")
                ps_yi = psy_pool.tile([40, E], F32, name="ps_yi")
                for c in range(NCH):
                    w = w_sb[:, c, :]
                    for h in range(2):
                        fs = slice(h * 512, (h + 1) * 512)
                        nc.tensor.matmul(
                            ps_yr[:, fs],
                            w,
                            tkr_c(c)[:, fs],
                            start=(c == 0),
                            stop=(c == NCH - 1),
                        )
                        nc.tensor.matmul(
                            ps_yi[:, fs],
                            w,
                            tki_c(c)[:, fs],
                            start=(c == 0),
                            stop=(c == NCH - 1),
                        )
                yr_sb = evacpool.tile([40, E], F32, name="yr_sb")
                nc.vector.tensor_copy(yr_sb[:], ps_yr[:])
                nc.gpsimd.dma_start(yr_d[b], yr_sb[:])
                yi_sb = evacpool.tile([40, E], F32, name="yi_sb")
                nc.vector.tensor_copy(yi_sb[:], ps_yi[:])
                nc.gpsimd.dma_start(yi_d[b], yi_sb[:])

                # ---- den partials: sum of squares along free dim (fp32
                # accumulate). Alternate chunks between DVE and ACT so the
                # straggler work after the last DMA splits across engines.
                for c in range(NCH):
                    col = b * NCH + c
                    for mat, src in ((0, tkr_c(c)), (1, tki_c(c))):
                        if (c + mat) % 2 == 0:
                            scr_v = scrpool.tile([128, E], F16, name="scr_v")
                            nc.vector.scalar_tensor_tensor(
                                scr_v[:],
                                src,
                                1.0,
                                src,
                                mult,
                                mult,
                                accum_out=den_dve[:, col:col + 1],
                            )
                        else:
                            scr_a = scrpool.tile([128, E], F16, name="scr_a")
                            nc.scalar.activation(
                                scr_a[:],
                                src,
                                Square,
                                accum_out=den_act[:, col:col + 1],
                            )

            nc.gpsimd.dma_start(den_d[0], den_dve[:])
            nc.gpsimd.dma_start(den_d[1], den_act[:])

    nc.compile()
    return nc


def _get_program():
    if "nc" not in _PROGRAM_CACHE:
        _PROGRAM_CACHE["nc"] = _build_program()
    return _PROGRAM_CACHE["nc"]


def _pack_inputs(nn, tkr, tki):
    """Host-side packing: per-core input dicts with device-friendly layouts."""
    tkr16 = tkr.astype(np.float16)
    tki16 = tki.astype(np.float16)
    # [B, E, K] slices of nn
    Ur = nn[:, K:VLOC].reshape(B, E, K)
    Ui = nn[:, NOUT + K:NOUT + VLOC].reshape(B, E, K)
    Vr = nn[:, VLOC:NOUT].reshape(B, E, K)
    Vi = nn[:, NOUT + VLOC:2 * NOUT].reshape(B, E, K)
    xuv = np.concatenate([Ur, Ui, Vr, Vi], axis=2)        # [B, E, 80] f32
    # partition-major: e = c*128 + p  ->  [B, p, c, 80]
    xuv = np.ascontiguousarray(
        xuv.reshape(B, NCH, 128, 80).transpose(0, 2, 1, 3)
    )
    w16 = np.ascontiguousarray(
        np.concatenate([Ur, Ui], axis=2)
        .reshape(B, NCH, 128, 40)
        .transpose(0, 2, 1, 3)
        .astype(np.float16)
    )
    return [
        {
            "xuv": xuv[i * NB:(i + 1) * NB],
            "w16": w16[i * NB:(i + 1) * NB],
            "tkr": tkr16[i * NB:(i + 1) * NB],
            "tki": tki16[i * NB:(i + 1) * NB],
        }
        for i in range(NCORES)
    ]


def _run_device(nn, tkr, tki, trace=False):
    nc = _get_program()
    in_maps = _pack_inputs(nn, tkr, tki)
    return run_bass_kernel_spmd(nc, in_maps, list(range(NCORES)), trace=trace)


def _finalize(nn, results, batch_size):
    """Assemble (loss, obj1, obj2) from per-core device partials (float64)."""
    nn = np.asarray(nn)
    d = (nn[:, :K] + 1j * nn[:, NOUT:NOUT + K]).astype(np.complex128)
    Vr = nn[:, VLOC:NOUT].reshape(B, E, K).astype(np.float64)
    Vi = nn[:, NOUT + VLOC:2 * NOUT].reshape(B, E, K).astype(np.float64)
    V = Vr + 1j * Vi

    gram = np.concatenate(
        [r["gram"] for r in results], axis=0
    ).astype(np.float64)                                   # [B, 40, 80]
    yr = np.concatenate([r["yr"] for r in results], axis=0).astype(np.float64)
    yi = np.concatenate([r["yi"] for r in results], axis=0).astype(np.float64)
    den = float(sum(np.sum(r["den"], dtype=np.float64) for r in results))

    SU = gram[:, :, 0:40]
    SV = gram[:, :, 40:80]
    Srr = SU[:, 0:20, 0:20]
    Sri = SU[:, 0:20, 20:40]
    Sii = SU[:, 20:40, 20:40]
    Trr = SV[:, 0:20, 0:20]
    Tri = SV[:, 0:20, 20:40]
    Tii = SV[:, 20:40, 20:40]
    SriT = np.transpose(Sri, (0, 2, 1))
    TriT = np.transpose(Tri, (0, 2, 1))
    G_U = (Srr - Sii) + 1j * (Sri + SriT)
    G_V = (Trr - Tii) + 1j * (Tri + TriT)
    H_U = (Srr + Sii) + 1j * (Sri - SriT)
    H_V = (Trr + Tii) + 1j * (Tri - TriT)

    mask = np.triu(np.ones((K, K), dtype=bool), k=1)
    bsz = float(batch_size)
    obj1 = float(np.sum(np.abs(G_U)[:, mask]) / bsz)
    obj2 = float(np.sum(np.abs(G_V)[:, mask]) / bsz)

    prednorm = float(
        np.real(
            np.einsum("bk,bl,bkl,bkl->", d, np.conj(d), np.conj(H_U), np.conj(H_V))
        )
    )

    # cross = Re<conj(tk), pred>; Wc[b,k,f] = sum_e conj(tk[e,f]) U[e,k]
    Wc = (yr[:, 0:20, :] + yi[:, 20:40, :]) + 1j * (yr[:, 20:40, :] - yi[:, 0:20, :])
    zeta = np.einsum("bfk,bkf->bk", V, Wc)
    cross = float(np.real(np.einsum("bk,bk->", d, zeta)))

    num = den - 2.0 * cross + prednorm
    loss = num / den + PENALTY * (obj1 + obj2)
    return (
        np.float32(loss),
        np.float32(obj1),
        np.float32(obj2),
    )


def kernel(nnOutput, kern_real, kern_imag, batch_Size):
    nn = np.ascontiguousarray(np.asarray(nnOutput, dtype=np.float32))
    tkr = np.asarray(kern_real, dtype=np.float32)
    tki = np.asarray(kern_imag, dtype=np.float32)
    res = _run_device(nn, tkr, tki).results
    return _finalize(nn, res, int(batch_Size))
